# revision 11
# baseline (speedup 1.0000x reference)
"""Trainium2 Bass kernel: BinarizeLinear inference.

Computes out = sign01(x) @ weight + bias where sign01(t) = +1 if t > 0 else -1,
for x [8192, 4096] f32, weight [4096, 4096] f32, bias [4096] f32.

Strategy: data-parallel over the token dim across 8 NeuronCores (each core
gets x[c*1024:(c+1)*1024], the full weight, and the bias). No collectives;
outputs are concatenated on the host.

Per-core kernel:
  - x arrives host-transposed [k, m_shard]; binarize to {+1,-1} on the
    Scalar engine (Sign activation with a tiny negative bias so exact zeros
    map to -1) straight into resident k-major tiles xbt (fp8e4 by default:
    +-1 is exact in fp8, stationary dtype only affects LDWEIGHTS; 32 KB/par),
  - stream weight once as column-group tiles [128p, kt_h, NTI*512n] bf16
    (f32->bf16 cast in-flight via SWDGE cast-DMA), k-halves double-buffered,
  - matmul in stationary-REUSE order: each stationary xbt[:, kt, m_sl] is
    loaded once and streams the group's NTI column chunks back-to-back into
    NTI PSUM banks (measured substantially faster per MM than rotating the
    stationary every matmul — LDWEIGHTS amortizes),
  - accumulate out tiles [128m, 512n] in PSUM over 32 k-tiles,
  - evict with a fused bias add on the Vector engine, DMA to DRAM.
"""

import contextlib
import os
import sys

import numpy as np

os.environ.setdefault("JAX_PLATFORMS", "axon")

for _p in ("/opt/trn_rl_repo", "/root/.axon_site/_ro/trn_rl_repo"):
    if os.path.isdir(_p) and _p not in sys.path:
        sys.path.insert(0, _p)
        break

import concourse.bass as bass  # noqa: E402
import concourse.mybir as mybir  # noqa: E402
import concourse.tile as tile  # noqa: E402
from concourse import bacc  # noqa: E402
from concourse.bass_utils import run_bass_kernel_spmd  # noqa: E402

P = 128
N_CORES = 8
TOKENS, IN_F, OUT_F = 8192, 4096, 4096
F32 = mybir.dt.float32
BF16 = mybir.dt.bfloat16

# number of n-chunks sharing one stationary load (PSUM banks per mt)
NTI = int(os.environ.get("BINLIN_NTI", "2"))
# k-slices per W group tile (each [P, kt_n/KH, NTI*n_chunk], double-buffered)
KH = int(os.environ.get("BINLIN_KH", "2"))
# xbt (binarized activations, stationary operand) dtype
XDT = os.environ.get("BINLIN_XDT", "float8e4")
# xbt tile split along m (early matmul start while binarize continues)
XSPLIT = int(os.environ.get("BINLIN_XSPLIT", "4"))
# timing-attribution probes (NEVER set when grading): no_binarize, no_wdma,
# no_evict, no_mm — each removes one subsystem, keeping the dep structure
PROBE = set(filter(None, os.environ.get("BINLIN_PROBE", "").split(",")))
# split each chunk's kt accumulation across this many PSUM banks (deeper
# bank alternation lets consecutive matmuls pipeline; extra DVE add on evict)
KSPLIT = int(os.environ.get("BINLIN_KSPLIT", "1"))


def build_nc(
    m_shard=TOKENS // N_CORES,
    k=IN_F,
    n=OUT_F,
    n_chunk=512,
    loop_k=1,
    nti=None,
    kh=None,
    xdt=None,
    xsplit=None,
    ksplit=None,
):
    """loop_k > 1 wraps the whole body in a hardware For loop that repeats
    the identical computation; used only for wall-clock slope timing."""
    nti = nti or NTI
    kh = kh or KH
    ksplit = ksplit or KSPLIT
    xdt_m = getattr(mybir.dt, xdt or XDT)
    xsplit = xsplit or XSPLIT
    mt_n = m_shard // P
    kt_n = k // P
    nt_n = n // n_chunk
    assert m_shard % P == 0 and k % P == 0 and n % n_chunk == 0
    assert nt_n % nti == 0 and kt_n % kh == 0 and mt_n % xsplit == 0
    kt_h = kt_n // kh  # k-tiles per W group slice
    m_sp = m_shard // xsplit  # tokens per xbt tile
    mt_sp = mt_n // xsplit

    nc = bacc.Bacc(
        "TRN2", target_bir_lowering=False, debug=False, num_devices=N_CORES
    )
    # x arrives host-transposed as [k, m_shard]
    x_ap = nc.declare_dram_parameter("x", [k, m_shard], F32, isOutput=False).ap()
    w_ap = nc.declare_dram_parameter("weight", [k, n], F32, isOutput=False).ap()
    b_ap = nc.declare_dram_parameter("bias", [P, n], F32, isOutput=False).ap()
    out_ap = nc.declare_dram_parameter("out", [m_shard, n], F32, isOutput=True).ap()
    # weight rows k = kt*P + p -> [p, kt, n]
    w_t = w_ap.rearrange("(kt p) n -> p kt n", p=P)
    xt_t = x_ap.rearrange("(kt p) m -> p kt m", p=P)

    with tile.TileContext(nc) as tc:
        with (
            tc.tile_pool(name="const", bufs=1) as const_pool,
            tc.tile_pool(name="xbt", bufs=1) as xbt_pool,
            tc.tile_pool(name="xrh", bufs=6) as xrh_pool,
            tc.tile_pool(name="wslot", bufs=2) as w_pool,
            tc.tile_pool(name="osb", bufs=3) as o_pool,
            tc.tile_pool(name="mm_psum", bufs=2, space="PSUM") as mm_psum,
        ):
            if "no_evict" not in PROBE:
                bias_sb = const_pool.tile([P, n], F32)
                nc.sync.dma_start(bias_sb[:], b_ap[:, :])
            else:
                bias_sb = None
            # per-partition tiny negative bias for the sign-binarize
            sgn_bias = const_pool.tile([P, 1], F32)
            nc.gpsimd.memset(sgn_bias[:], -1e-30)

            loop_cm = (
                tc.For_i(0, loop_k, 1) if loop_k > 1 else contextlib.nullcontext()
            )
            with loop_cm:
                # ---- phase A: binarize x into resident k-major xbt tiles ----
                xbts = [
                    xbt_pool.tile([P, kt_n, m_sp], xdt_m, name=f"xb{s}")
                    for s in range(xsplit)
                ]
                if "no_binarize" in PROBE:
                    for s in range(xsplit):
                        nc.gpsimd.memset(xbts[s][:], 1.0)
                else:
                    MB = min(m_sp, 256)
                    for s in range(xsplit):
                        for mb in range(m_sp // MB):
                            m_bl = slice(mb * MB, (mb + 1) * MB)
                            m_src = slice(
                                s * m_sp + mb * MB, s * m_sp + (mb + 1) * MB
                            )
                            for kt in range(kt_n):
                                xr = xrh_pool.tile([P, MB], F32, name="xr")
                                nc.sync.dma_start(xr[:], xt_t[:, kt, m_src])
                                # sign(x-tiny): zeros -> -1 like where(x>0,1,-1)
                                nc.scalar.sign(
                                    xbts[s][:, kt, m_bl], xr[:],
                                    bias=sgn_bias[:],
                                )

                # ---- phase B: stream W once, matmul in reuse order ----
                n_grp = nti * n_chunk
                for g in range(nt_n // nti):
                    n_gsl = slice(g * n_grp, (g + 1) * n_grp)
                    slots = []
                    for h in range(kh):
                        wck = w_pool.tile(
                            [P, kt_h, n_grp], BF16, name=f"w{h}"
                        )
                        k_sl = slice(h * kt_h, (h + 1) * kt_h)
                        # SWDGE cast-DMA: f32 DRAM -> bf16 SBUF
                        if "no_wdma" not in PROBE or g == 0:
                            nc.gpsimd.dma_start(wck[:], w_t[:, k_sl, n_gsl])
                        else:
                            nc.gpsimd.memset(wck[:, 0:1, 0:16], 1.0)
                        slots.append(wck)
                    for mt in range(mt_n):
                        xb = xbts[mt // mt_sp]
                        m_off = (mt % mt_sp) * P
                        m_sl = slice(mt * P, (mt + 1) * P)
                        pss = [
                            [
                                mm_psum.tile(
                                    [P, n_chunk], F32, name=f"ps{c}_{a}"
                                )
                                for a in range(ksplit)
                            ]
                            for c in range(nti)
                        ]
                        if "no_mm" not in PROBE:
                            for kt in range(kt_n):
                                h, kr = kt // kt_h, kt % kt_h
                                a = kt % ksplit
                                for c in range(nti):
                                    nc.tensor.matmul(
                                        pss[c][a][:],
                                        xb[:, kt, m_off : m_off + P],
                                        slots[h][
                                            :, kr,
                                            c * n_chunk : (c + 1) * n_chunk,
                                        ],
                                        start=(kt < ksplit),
                                        stop=(kt >= kt_n - ksplit),
                                    )
                        else:
                            for c in range(nti):
                                for a in range(ksplit):
                                    nc.tensor.matmul(
                                        pss[c][a][:],
                                        xb[:, 0, m_off : m_off + P],
                                        slots[0][
                                            :, 0, c * n_chunk : (c + 1) * n_chunk
                                        ],
                                        start=True, stop=True,
                                    )
                        if "no_evict" in PROBE and not (
                            g == nt_n // nti - 1 and mt == mt_n - 1
                        ):
                            continue
                        for c in range(nti):
                            ntc = g * nti + c
                            n_sl = slice(ntc * n_chunk, (ntc + 1) * n_chunk)
                            osb = o_pool.tile([P, n_chunk], F32, name=f"o{c}")
                            if bias_sb is None:
                                nc.vector.tensor_copy(osb[:], pss[c][0][:])
                            else:
                                nc.vector.tensor_add(
                                    osb[:], pss[c][0][:], bias_sb[:, n_sl]
                                )
                            for a in range(1, ksplit):
                                nc.vector.tensor_tensor(
                                    osb[:], osb[:], pss[c][a][:],
                                    mybir.AluOpType.add,
                                )
                            nc.sync.dma_start(out_ap[m_sl, n_sl], osb[:])

    nc.compile()
    return nc


_NC_CACHE = {}


def _get_nc(cfg):
    nc = _NC_CACHE.get(cfg)
    if nc is None:
        nc = _NC_CACHE[cfg] = build_nc(*cfg)
    return nc


def kernel(x, weight, bias, _trace=False):
    x = np.ascontiguousarray(np.asarray(x, dtype=np.float32))
    weight = np.ascontiguousarray(np.asarray(weight, dtype=np.float32))
    bias = np.ascontiguousarray(np.asarray(bias, dtype=np.float32))
    tokens, k = x.shape
    n = weight.shape[1]
    m_shard = tokens // N_CORES
    assert tokens % N_CORES == 0

    bias_b = np.ascontiguousarray(np.broadcast_to(bias[None, :], (P, n)))
    xt = np.ascontiguousarray(x.T)  # [k, tokens]
    x_shards = [
        np.ascontiguousarray(xt[:, c * m_shard : (c + 1) * m_shard])
        for c in range(N_CORES)
    ]
    in_maps = [
        {"x": x_shards[c], "weight": weight, "bias": bias_b}
        for c in range(N_CORES)
    ]
    nc = _get_nc((m_shard, k, n, 512, 1))
    res = run_bass_kernel_spmd(nc, in_maps, list(range(N_CORES)), trace=_trace)
    out = np.concatenate([res.results[c]["out"] for c in range(N_CORES)], axis=0)
    if _trace:
        return out, res
    return out
